# revision 15
# baseline (speedup 1.0000x reference)
"""BatchChildSumTreeLSTM Trainium2 kernel (8 NeuronCores, SPMD).

Strategy: data-parallel over trees (512 trees/core, contiguous per-level
slices). Per level, host compacts the embedding table per (core, level) via
np.unique so indices fit int16 for dma_gather(transpose=True), which lands
gathered embeddings directly feature-major in SBUF. All PE matmuls in bf16
(fp32 PSUM accumulate); child-sum h-side via strided-rhs accumulating
matmuls; forget-gate parent broadcast via step-0 rhs access patterns.
Levels 5->4->3->2 are chunk-fused (their h,c never leave SBUF); levels
2..0 are SBUF-resident full-level (bf16). Single kernel, no DRAM spill.
"""
import sys, os

for _p in ("/opt/trn_rl_repo", "/root/.axon_site/_ro/trn_rl_repo"):
    if os.path.isdir(_p) and _p not in sys.path:
        sys.path.append(_p)

import numpy as np
import ml_dtypes

BF = ml_dtypes.bfloat16

# ---- problem constants (hardcoded per contract) ----
LEVEL_SIZES = [4096, 16384, 65536, 131072, 262144, 262144]
OFF = [0]
for s in LEVEL_SIZES:
    OFF.append(OFF[-1] + s)
N_NODES = OFF[-1]
VOCAB = 50000
D = 128
NCORES = 8
NL = [s // NCORES for s in LEVEL_SIZES]   # [512, 2048, 8192, 16384, 32768, 32768]
RC = [4, 4, 2, 2, 1]                      # children per node, levels 0..4
TABR = list(NL)                           # padded table rows per level

C_A = 2048                                # stage-A chunk (level 5/4 cols)

_nc_cache = {}


def _patch_swdge_lane_assignment():
    """Pin each SWDGE DMA's completion-sem lane to its queue_num so Tile's
    scheduled-order round-robin can't pair a lane with two queues (the sim/HW
    lock a DMASW sem to one SWDGE queue)."""
    import concourse.tile_sem_assignment as tsa
    if getattr(tsa, "_lane_by_queue_patch", False):
        return
    import concourse.mybir as mybir
    import concourse.bass_isa as bass_isa
    orig = tsa.TileClockTick._assign_tick

    def _assign_tick(self, inst):
        if (isinstance(inst, tsa.DMAInst)
                and not isinstance(inst, bass_isa.UserSyncedRemoteDMADescs)
                and inst.engine == mybir.EngineType.Pool):
            q = getattr(inst, "queue_num", None)
            if q is not None:
                # steer the round-robin so lane == queue_num for this inst
                self.next_sw_dma_idx = q % self.swdge_sem_count
        orig(self, inst)

    tsa.TileClockTick._assign_tick_orig = orig
    tsa.TileClockTick._assign_tick = _assign_tick
    tsa._lane_by_queue_patch = True


def _build_nc():
    import concourse.mybir as mybir
    from concourse import bacc
    from concourse.tile import TileContext

    f32 = mybir.dt.float32
    bf16 = mybir.dt.bfloat16
    i16 = mybir.dt.int16
    AF = mybir.ActivationFunctionType

    nc = bacc.Bacc(num_swdge_queues=1)

    tabs = [nc.declare_dram_parameter(f"tab{L}", [TABR[L], D], bf16, isOutput=False)
            for L in range(6)]
    sens = [nc.declare_dram_parameter(f"sen{L}", [128, NL[L] // 16], i16, isOutput=False)
            for L in range(6)]
    Wg = {}
    for g in ("ix", "ih", "ox", "oh", "ux", "uh", "fx", "fh"):
        Wg[g] = nc.declare_dram_parameter(f"W_{g}", [D, D], bf16, isOutput=False)
    Wout = nc.declare_dram_parameter("W_out", [D, 4], bf16, isOutput=False)
    bias_in = nc.declare_dram_parameter("bias4", [128, 4], f32, isOutput=False)
    bout_in = nc.declare_dram_parameter("b_out", [4, 1], f32, isOutput=False)
    out_t = nc.declare_dram_parameter("out", [4, NL[0]], f32, isOutput=True)
    DEBUG = bool(int(os.environ.get("KDEBUG", "0")))
    dbg = {}
    if DEBUG:
        for name, nn_ in (("dbg_x5", 2048), ("dbg_h5", 2048), ("dbg_c5", 2048),
                          ("dbg_h4", 2048), ("dbg_c4", 2048), ("dbg_h3", 1024),
                          ("dbg_c3", 1024), ("dbg_h2", 512), ("dbg_c2", 512),
                          ("dbg_x2", 512), ("dbg_f3", 2048)):
            dbg[name] = nc.declare_dram_parameter(name, [128, nn_], mybir.dt.bfloat16,
                                                  isOutput=True)

    qn = [0]

    def nextq():
        return 0

    uid = [0]

    def nm(p):
        uid[0] += 1
        return f"{p}{uid[0]}"

    with TileContext(nc) as tc:
        with tc.tile_pool(name="cst", bufs=1) as cst, \
             tc.tile_pool(name="stat", bufs=1) as stat, \
             tc.tile_pool(name="xp", bufs=4) as xp_pool, \
             tc.tile_pool(name="gt", bufs=1) as gt, \
             tc.tile_pool(name="ps", bufs=2, space="PSUM") as ps:

            # ---- constants ----
            w = {}
            for g in Wg:
                w[g] = cst.tile([128, 128], bf16, tag=f"w_{g}", name=f"w_{g}")
                nc.sync.dma_start(out=w[g][:, :], in_=Wg[g][:, :])
            wout = cst.tile([128, 4], bf16)
            nc.sync.dma_start(out=wout[:, :], in_=Wout[:, :])
            bias = cst.tile([128, 4], f32)
            nc.sync.dma_start(out=bias[:, :], in_=bias_in[:, :])
            b_i, b_o, b_u, b_f = (bias[:, k:k + 1] for k in range(4))
            bout = cst.tile([4, 1], f32)
            nc.sync.dma_start(out=bout[:, :], in_=bout_in[:, :])
            sen_t = []
            for L in range(6):
                t = cst.tile([128, NL[L] // 16], i16, tag=f"sen{L}", name=f"sen{L}t")
                nc.sync.dma_start(out=t[:, :], in_=sens[L][:, :])
                sen_t.append(t)

            # ---- full-level statics (bf16): levels 2, 1, 0 ----
            h2 = stat.tile([128, NL[2]], bf16)
            c2 = stat.tile([128, NL[2]], bf16)
            h1 = stat.tile([128, NL[1]], bf16)
            c1 = stat.tile([128, NL[1]], bf16)
            h0 = stat.tile([128, NL[0]], bf16)

            def tile(n, tagp):
                bufs = {2048: 14, 1024: 8, 512: 10}[n]
                return gt.tile([128, n], bf16, tag=f"t{n}", name=nm(tagp), bufs=bufs)

            from concourse.tile import add_dep_helper
            prev_g = [None]

            def gather(L, c0, n):
                x = xp_pool.tile([128, n], bf16, tag=f"x_{n}", name=nm("x"))
                gi = nc.gpsimd.dma_gather(
                    out_ap=x[:, :].unsqueeze(1), in_ap=tabs[L][:, :],
                    idxs_ap=sen_t[L][:, c0 // 16:(c0 + n) // 16],
                    num_idxs=n, num_idxs_reg=n, elem_size=128, transpose=True,
                    single_packet=False, queue_num=nextq())
                gi_inst = getattr(gi, "instruction", None) or getattr(gi, "ins", None) or gi
                if prev_g[0] is not None:
                    add_dep_helper(gi_inst, prev_g[0], sync=False,
                                   reason="pin SWDGE order")
                prev_g[0] = gi_inst
                return x

            def level(x, hch, cch, r, n, hout=None, ho=0, cout=None, co=0):
                """Process n parent cols of one level.

                x: [128, n] gathered embeddings. hch/cch: child h/c APs
                [128, n*r] (None for leaves). h written to hout[:, ho:ho+n]
                (fresh tile if None); c likewise via cout/co.
                Returns (h_ap, c_ap).
                """
                gates = {}
                for g_x, g_h, b, fn, gname in (("ix", "ih", b_i, AF.Sigmoid, "i"),
                                               ("ox", "oh", b_o, AF.Sigmoid, "o"),
                                               ("ux", "uh", b_u, AF.Tanh, "u")):
                    pre = ps.tile([128, 2048], f32, tag="ps_g", name=nm("ps"))
                    for s0 in range(0, n, 512):
                        sn = min(512, n - s0)
                        mms = [(w[g_x], x[:, s0:s0 + sn])]
                        if hch is not None:
                            for k in range(r):
                                mms.append((w[g_h], hch[:, s0 * r + k:(s0 + sn) * r:r]))
                        for mi, (lw, rhs) in enumerate(mms):
                            nc.tensor.matmul(out=pre[:, s0:s0 + sn], lhsT=lw[:, :],
                                             rhs=rhs, start=(mi == 0),
                                             stop=(mi == len(mms) - 1))
                    res = tile(n, "g" + gname)
                    nc.scalar.activation(out=res[:, :], in_=pre[:, :n], func=fn, bias=b)
                    gates[gname] = res
                fc = None
                if hch is not None:
                    ncld = n * r
                    fpre = ps.tile([128, 2048], f32, tag="ps_g", name=nm("ps"))
                    for s0 in range(0, ncld, 512):
                        sn = min(512, ncld - s0)
                        pa, pn = s0 // r, sn // r
                        if r == 1:
                            xdup = x[:, pa:pa + sn]
                        else:
                            xdup = x[:, pa:pa + pn].unsqueeze(2).to_broadcast([128, pn, r])
                        nc.tensor.matmul(out=fpre[:, s0:s0 + sn], lhsT=w["fx"][:, :],
                                         rhs=xdup, start=True, stop=False)
                        nc.tensor.matmul(out=fpre[:, s0:s0 + sn], lhsT=w["fh"][:, :],
                                         rhs=hch[:, s0:s0 + sn], start=False, stop=True)
                    f = tile(ncld, "gf")
                    nc.scalar.activation(out=f[:, :], in_=fpre[:, :ncld],
                                         func=AF.Sigmoid, bias=b_f)
                    fc = tile(ncld, "fc")
                    nc.vector.tensor_mul(out=fc[:, :], in0=f[:, :], in1=cch)
                if cout is None:
                    cout = tile(n, "c")
                    co = 0
                cc = cout[:, co:co + n]
                nc.vector.tensor_mul(out=cc, in0=gates["i"][:, :], in1=gates["u"][:, :])
                if fc is not None:
                    for k in range(r):
                        nc.vector.tensor_add(out=cc, in0=cc, in1=fc[:, k::r])
                tcn = tile(n, "tc")
                nc.scalar.activation(out=tcn[:, :], in_=cc, func=AF.Tanh)
                if hout is None:
                    hout = tile(n, "h")
                    ho = 0
                nc.vector.tensor_mul(out=hout[:, ho:ho + n], in0=gates["o"][:, :],
                                     in1=tcn[:, :])
                return hout[:, ho:ho + n], cc

            # ---- ACT table warmup: force sigmoid/tanh spline tables resident
            # before the first real gate activation (table-load DMA races the
            # first ACT op on HW; CoreSim loads tables instantaneously) ----
            osb = cst.tile([4, NL[0]], f32, name="osb")
            awu = cst.tile([128, 128], f32, name="actwu")
            nc.scalar.activation(out=awu[:, :], in_=w["ix"][:, :], func=AF.Sigmoid)
            nc.scalar.activation(out=awu[:, :], in_=awu[:, :], func=AF.Tanh)

            # ---- SWDGE queue warmup: first gather on each queue can race
            # queue-ring init on HW; throw one tiny gather per queue away ----
            for q in range(4):
                gather(5, 0, 128)

            # ===== stage A: fused levels 5 -> 4 -> 3 -> 2 =====
            for ci in range(NL[5] // C_A):
                a5 = ci * C_A
                a3 = ci * (C_A // 2)
                a2 = ci * (C_A // 4)
                x5 = gather(5, a5, C_A)
                x4 = gather(4, a5, C_A)
                x3 = gather(3, a3, C_A // 2)
                x2 = gather(2, a2, C_A // 4)
                h5, c5 = level(x5, None, None, 0, C_A)
                h4, c4 = level(x4, h5, c5, 1, C_A)
                h3, c3 = level(x3, h4, c4, 2, C_A // 2)
                level(x2, h3, c3, 2, C_A // 4, hout=h2, ho=a2, cout=c2, co=a2)
                if DEBUG and ci == 0:
                    for name, ap in (("dbg_x5", x5[:, :]), ("dbg_h5", h5), ("dbg_c5", c5),
                                     ("dbg_h4", h4), ("dbg_c4", c4), ("dbg_h3", h3),
                                     ("dbg_c3", c3), ("dbg_x2", x2[:, :]),
                                     ("dbg_h2", h2[:, a2:a2 + 512]),
                                     ("dbg_c2", c2[:, a2:a2 + 512])):
                        nc.sync.dma_start(out=dbg[name][:, :], in_=ap)

            # ===== stage B: level 1 (children = level-2 statics, r=4) =====
            for pc in range(0, NL[1], 512):
                x1c = gather(1, pc, 512)
                level(x1c, h2[:, pc * 4:(pc + 512) * 4], c2[:, pc * 4:(pc + 512) * 4],
                      4, 512, hout=h1, ho=pc, cout=c1, co=pc)

            # ===== stage C: level 0 (children = level-1 statics, r=4) =====
            x0 = gather(0, 0, NL[0])
            level(x0, h1[:, :], c1[:, :], 4, NL[0], hout=h0, ho=0)

            # ===== output: logits = W_out.T @ h0 + b_out =====
            opre = ps.tile([4, 512], f32, tag="ps_g", name=nm("po"))
            nc.tensor.matmul(out=opre[:, :], lhsT=wout[:, :], rhs=h0[:, :],
                             start=True, stop=True)
            nc.scalar.activation(out=osb[:, :], in_=opre[:, :NL[0]], func=AF.Identity,
                                 bias=bout[:, :])
            nc.sync.dma_start(out=out_t[:, :], in_=osb[:, :])

    nc.finalize()
    return nc


def _get_nc():
    if "nc" not in _nc_cache:
        _nc_cache["nc"] = _build_nc()
    return _nc_cache["nc"]


def _pack_idx(inv):
    """int32 inverse indices -> [128, n/16] int16 wrap layout replicated 8x."""
    n = inv.shape[0]
    blk = np.zeros((16, n // 16), np.uint16)
    blk[np.arange(n) % 16, np.arange(n) // 16] = inv.astype(np.uint16)
    return np.tile(blk, (8, 1)).view(np.int16)


def _prep_core(k, sen, emb_bf):
    m = {}
    for L in range(6):
        n8 = NL[L]
        base = OFF[L] + k * n8
        ids = sen[base:base + n8]
        uniq, inv = np.unique(ids, return_inverse=True)
        tab = np.zeros((TABR[L], D), BF)
        tab[:uniq.shape[0]] = emb_bf[uniq]
        m[f"tab{L}"] = tab
        m[f"sen{L}"] = _pack_idx(inv.astype(np.int32))
    return m


def _make_in_maps(inputs):
    sen = np.asarray(inputs["sen"])
    emb_bf = np.asarray(inputs["embedding"]).astype(BF)
    w = {f"W_{g}": np.asarray(inputs[f"W_{g}"]).astype(BF)
         for g in ("ix", "ih", "ox", "oh", "ux", "uh", "fx", "fh")}
    bias4 = np.stack([
        np.asarray(inputs["b_ix"]) + np.asarray(inputs["b_ih"]),
        np.asarray(inputs["b_ox"]) + np.asarray(inputs["b_oh"]),
        np.asarray(inputs["b_ux"]) + np.asarray(inputs["b_uh"]),
        np.asarray(inputs["b_fx"]) + np.asarray(inputs["b_fh"]),
    ], axis=1).astype(np.float32)                       # [128, 4]
    wout = np.asarray(inputs["W_out"]).astype(BF)
    bout = np.asarray(inputs["b_out"]).astype(np.float32).reshape(4, 1)
    in_maps = []
    for k in range(NCORES):
        m = _prep_core(k, sen, emb_bf)
        m.update(w)
        m["W_out"] = wout
        m["bias4"] = bias4
        m["b_out"] = bout
        in_maps.append(m)
    return in_maps


def _run(inputs, trace=False, tmpdir=None):
    from concourse.bass_utils import run_bass_kernel_spmd
    nc = _get_nc()
    in_maps = _make_in_maps(inputs)
    res = run_bass_kernel_spmd(nc, in_maps, core_ids=list(range(NCORES)),
                               trace=trace, tmpdir=tmpdir)
    outs = []
    for k in range(NCORES):
        o = np.asarray(res.results[k]["out"], dtype=np.float32)   # [4, 512]
        outs.append(o.T)                                          # [512, 4]
    return np.concatenate(outs, axis=0), res                      # [4096, 4]


def kernel(**inputs) -> np.ndarray:
    out, _ = _run(inputs, trace=False)
    return out
